# revision 14
# baseline (speedup 1.0000x reference)
"""Distributed 2-layer GCN + mean-pool + FC for Trainium2 (8 NeuronCores).

Sharding: nodes are partitioned contiguously at graph boundaries across the 8
cores; each core owns the edges whose *destination* lies in its node range.
Small weights are replicated. Per GCN layer each core computes its shard of
the scaled node features h' = dinv * (h @ W), the shards are AllGathered, and
each core aggregates its incident edges locally:

    agg[dst] = sum_{(s,d) edges, d local} h'_full[s]
    out[dst] = relu(dinv[dst] * agg[dst] + b)

The edge aggregation runs on the TensorEngine: for each 128-edge chunk, the
source rows are fetched with dma_gather (G, [128 edges x 128 feats]) and a
one-hot dst-slot matrix S ([128 edges x 128 slots]) is built on the vector
engine by comparing an iota row against the per-edge dst slot; a single
matmul accumulates G^T @ S (layer 1, transposed aggregate) or S^T @ G
(layer 2) into PSUM per 128-node window. Mean-pool + FC use the same
one-hot-matmul trick over graph ids, entirely core-local.
"""

import math

import numpy as np
import ml_dtypes

P = 128
NCORES = 8
NRANGE = 4  # gather source split (int16 index limit)
BLK = 8     # windows per gather block

BF16 = ml_dtypes.bfloat16


def _ceil_to(x, m):
    return (x + m - 1) // m * m


# ---------------------------------------------------------------- host prep
def _preprocess(x, edge_index, batch, W1, b1, W2, b2, Wfc, bfc):
    N, IN_DIM = x.shape
    E = edge_index.shape[1]
    G = int(batch.max()) + 1 if batch.size else 1
    HID = W1.shape[1]
    OUT = Wfc.shape[1]
    assert HID == 128
    KPAD = _ceil_to(IN_DIM, P)
    KCH = KPAD // P

    batch = batch.astype(np.int64)
    src = np.concatenate([edge_index[0].astype(np.int64), np.arange(N)])
    dst = np.concatenate([edge_index[1].astype(np.int64), np.arange(N)])
    deg = np.bincount(dst, minlength=N)
    dinv = (1.0 / np.sqrt(np.maximum(deg, 1))).astype(np.float32)

    # node boundaries at graph starts
    graph_start = np.searchsorted(batch, np.arange(G + 1))
    nb = np.zeros(NCORES + 1, np.int64)
    nb[NCORES] = N
    for c in range(1, NCORES):
        g = batch[min(c * N // NCORES, N - 1)]
        nb[c] = graph_start[g]
    assert np.all(np.diff(nb) > 0)
    gb = np.concatenate([batch[nb[:NCORES]], [G]]).astype(np.int64)

    n_per = np.diff(nb)
    N_PAD = _ceil_to(int(n_per.max()), 512)
    assert 2 * N_PAD <= 32768
    W = N_PAD // P
    RANGE_ROWS = 2 * N_PAD
    g_per = np.diff(gb)
    assert g_per.max() <= P

    # per-edge owner / local coords
    eo = np.searchsorted(nb, dst, side="right") - 1
    dstloc = dst - nb[eo]
    so = np.searchsorted(nb, src, side="right") - 1
    srow = so * N_PAD + (src - nb[so])  # row in allgathered feature table
    er = srow // RANGE_ROWS
    ew = dstloc >> 7
    eslot = dstloc & 127
    eblk = ew // BLK
    eidx16 = (srow - er * RANGE_ROWS).astype(np.int64)
    assert eidx16.max() < 32768

    # cell (w, r) chunk counts, maxed across cores for the SPMD-uniform graph
    key = ((eo * W + ew) * NRANGE + er).astype(np.int64)
    counts = np.bincount(key, minlength=NCORES * W * NRANGE).reshape(
        NCORES, W, NRANGE
    )
    cell_chunks = -(-counts.max(axis=0) // P)  # [W, NRANGE]
    cell_chunks[:, 0] = np.maximum(cell_chunks[:, 0], 1)  # every window >=1 chunk

    # chunk layout: block -> r -> w -> chunk
    NBLK = -(-W // BLK)
    blocks = [list(range(b * BLK, min((b + 1) * BLK, W))) for b in range(NBLK)]
    cell_start = {}
    call_start = np.zeros((NBLK, NRANGE), np.int64)
    call_chunks = np.zeros((NBLK, NRANGE), np.int64)
    ch = 0
    for b, wl in enumerate(blocks):
        for r in range(NRANGE):
            call_start[b, r] = ch
            for w in wl:
                cell_start[(w, r)] = ch
                ch += cell_chunks[w, r]
            call_chunks[b, r] = ch - call_start[b, r]
    CH = ch

    # per-core tables
    cores = []
    for c in range(NCORES):
        m = eo == c
        n_c = int(n_per[c])
        g_c = int(g_per[c])
        cw, cr, cblk = ew[m], er[m], eblk[m]
        cidx, cslot = eidx16[m], eslot[m]
        order = np.lexsort((cidx, cw, cr, cblk))
        cw, cr = cw[order], cr[order]
        cidx, cslot = cidx[order], cslot[order]
        ccnt = counts[c]

        idx_flat = np.zeros(CH * P, np.int16)
        slot_flat = np.full(CH * P, -1.0, np.float32)
        ptr = 0
        for b, wl in enumerate(blocks):
            for r in range(NRANGE):
                for w in wl:
                    n = int(ccnt[w, r])
                    s = cell_start[(w, r)] * P
                    idx_flat[s : s + n] = cidx[ptr : ptr + n]
                    slot_flat[s : s + n] = cslot[ptr : ptr + n]
                    ptr += n
        assert ptr == int(m.sum())

        gidx = np.tile(
            idx_flat.reshape(CH * 8, 16).T, (8, 1)
        )  # [128, CH*8] int16 wrapped
        dslot = slot_flat.reshape(CH, P).T.copy()  # [128, CH] f32

        # node-feature shard, transposed+padded: [KCH, 128, N_PAD]
        xp = np.zeros((N_PAD, KPAD), np.float32)
        xp[:n_c, :IN_DIM] = x[nb[c] : nb[c + 1]]
        x3 = (
            np.ascontiguousarray(xp.T.reshape(KCH, P, N_PAD)).astype(BF16)
        )

        dl = np.zeros(N_PAD, np.float32)
        dl[:n_c] = dinv[nb[c] : nb[c + 1]]
        dinvc = dl.reshape(W, P).T.copy()  # [128, W] f32 (per-partition scalars)
        dinvb = np.tile(dl[None, :], (P, 1))  # [128, N_PAD] f32 (free-axis bcast)

        bl = np.full(N_PAD, -1.0, np.float32)
        bl[:n_c] = (batch[nb[c] : nb[c + 1]] - gb[c]).astype(np.float32)
        bslot = bl.reshape(W, P).T.copy()  # [128, W] f32

        cnt = np.bincount(
            (batch[nb[c] : nb[c + 1]] - gb[c]).astype(np.int64), minlength=P
        ).astype(np.float32)
        icnt = (1.0 / np.maximum(cnt, 1.0))[:, None].astype(np.float32)  # [128,1]

        cores.append(
            dict(
                x3=x3, gidx=gidx, dslot=dslot, dinvc=dinvc, dinvb=dinvb,
                bslot=bslot, icnt=icnt, n_c=n_c, g_c=g_c,
            )
        )

    # replicated tables
    W1p = np.zeros((KPAD, HID), np.float32)
    W1p[:IN_DIM] = W1
    w1r = np.ascontiguousarray(W1p.reshape(KCH, P, HID)).astype(BF16)
    w2 = W2.astype(BF16)
    wfc = Wfc.astype(BF16)
    b1c = b1[:, None].astype(np.float32)
    b2b = np.tile(b2[None, :], (P, 1)).astype(BF16)  # [128, HID]
    bfcb = np.tile(bfc[None, :], (P, 1)).astype(np.float32)  # [128, OUT]
    iota = np.tile(np.arange(P, dtype=np.float32), (P, 1)).astype(BF16)

    meta = dict(
        N=N, E=E, G=G, HID=HID, OUT=OUT, KCH=KCH, N_PAD=N_PAD, W=W,
        RANGE_ROWS=RANGE_ROWS, NBLK=NBLK, blocks=blocks, CH=CH,
        cell_chunks=cell_chunks, cell_start=cell_start,
        call_start=call_start, call_chunks=call_chunks,
        nb=nb, gb=gb,
    )
    shared = dict(w1r=w1r, w2=w2, wfc=wfc, b1c=b1c, b2b=b2b, bfcb=bfcb, iota=iota)
    return meta, shared, cores


# ---------------------------------------------------------------- device
def _build(meta):
    import concourse.bass as bass
    import concourse.mybir as mybir
    import concourse.tile as tile
    from concourse import bacc

    HID, OUT, KCH = meta["HID"], meta["OUT"], meta["KCH"]
    N_PAD, W, CH = meta["N_PAD"], meta["W"], meta["CH"]
    RANGE_ROWS, blocks = meta["RANGE_ROWS"], meta["blocks"]
    cell_chunks, cell_start = meta["cell_chunks"], meta["cell_start"]
    call_start, call_chunks = meta["call_start"], meta["call_chunks"]

    import os
    stop = os.environ.get("GCN_STOP", "")
    part = os.environ.get("GCN_PART", "full")
    dt = mybir.dt
    nc = bacc.Bacc(
        "TRN2",
        target_bir_lowering=False,
        debug=False,
        num_devices=NCORES,
    )
    rg = [list(range(NCORES))]

    # --- dram parameters
    x3_d = nc.declare_dram_parameter("x3", [KCH, P, N_PAD], dt.bfloat16, isOutput=False)
    w1r_d = nc.declare_dram_parameter("w1r", [KCH, P, HID], dt.bfloat16, isOutput=False)
    w2_d = nc.declare_dram_parameter("w2", [P, HID], dt.bfloat16, isOutput=False)
    wfc_d = nc.declare_dram_parameter("wfc", [P, OUT], dt.bfloat16, isOutput=False)
    b1c_d = nc.declare_dram_parameter("b1c", [P, 1], dt.float32, isOutput=False)
    b2b_d = nc.declare_dram_parameter("b2b", [P, HID], dt.bfloat16, isOutput=False)
    bfcb_d = nc.declare_dram_parameter("bfcb", [P, OUT], dt.float32, isOutput=False)
    icnt_d = nc.declare_dram_parameter("icnt", [P, 1], dt.float32, isOutput=False)
    iota_d = nc.declare_dram_parameter("iota", [P, P], dt.bfloat16, isOutput=False)
    dinvc_d = nc.declare_dram_parameter("dinvc", [P, W], dt.float32, isOutput=False)
    dinvb_d = nc.declare_dram_parameter("dinvb", [P, N_PAD], dt.float32, isOutput=False)
    bslot_d = nc.declare_dram_parameter("bslot", [P, W], dt.float32, isOutput=False)
    dslot_d = nc.declare_dram_parameter("dslot", [P, CH], dt.float32, isOutput=False)
    gidx_d = nc.declare_dram_parameter("gidx", [P, CH * 8], dt.int16, isOutput=False)
    out_d = nc.declare_dram_parameter("out", [P, OUT], dt.float32, isOutput=True)

    # --- internal dram
    ag1_in = nc.dram_tensor("ag1_in", [N_PAD, HID], dt.bfloat16)
    h1_full = nc.dram_tensor(
        "h1_full", [NCORES * N_PAD, HID], dt.bfloat16, addr_space="Shared"
    )
    ag2_in = nc.dram_tensor("ag2_in", [N_PAD, HID], dt.bfloat16)
    h2_full = nc.dram_tensor(
        "h2_full", [NCORES * N_PAD, HID], dt.bfloat16, addr_space="Shared"
    )

    with tile.TileContext(nc, num_cores=NCORES) as tc:
        with (
            tc.tile_pool(name="const", bufs=1) as cp,
            tc.tile_pool(name="tabs", bufs=1) as tp,
            tc.tile_pool(name="xin", bufs=2) as xp,
            tc.tile_pool(name="gath", bufs=2) as gp,
            tc.tile_pool(name="sbuild", bufs=8) as sp,
            tc.tile_pool(name="epi", bufs=4) as ep,
            tc.tile_pool(name="dinvbp", bufs=2) as dp,
            tc.tile_pool(name="psA", bufs=2, space="PSUM") as psA,
            tc.tile_pool(name="psAgg", bufs=2, space="PSUM") as psAgg,
            tc.tile_pool(name="psMisc", bufs=2, space="PSUM") as psM,
            tc.tile_pool(name="psPool", bufs=1, space="PSUM") as psP,
        ):
            # constants
            w1_sb = [cp.tile([P, HID], dt.bfloat16, tag=f"w1_{k}", name=f"w1_{k}") for k in range(KCH)]
            for k in range(KCH):
                nc.sync.dma_start(w1_sb[k][:], w1r_d[k])
            w2_sb = cp.tile([P, HID], dt.bfloat16)
            nc.sync.dma_start(w2_sb[:], w2_d[:])
            wfc_sb = cp.tile([P, OUT], dt.bfloat16)
            nc.sync.dma_start(wfc_sb[:], wfc_d[:])
            b1_sb = cp.tile([P, 1], dt.float32)
            nc.sync.dma_start(b1_sb[:], b1c_d[:])
            b2b_sb = cp.tile([P, HID], dt.bfloat16)
            nc.sync.dma_start(b2b_sb[:], b2b_d[:])
            bfcb_sb = cp.tile([P, OUT], dt.float32)
            nc.sync.dma_start(bfcb_sb[:], bfcb_d[:])
            icnt_sb = cp.tile([P, 1], dt.float32)
            nc.sync.dma_start(icnt_sb[:], icnt_d[:])
            iota_sb = cp.tile([P, P], dt.bfloat16)
            nc.sync.dma_start(iota_sb[:], iota_d[:])
            dinvc_sb = cp.tile([P, W], dt.float32)
            nc.sync.dma_start(dinvc_sb[:], dinvc_d[:])
            bslot_sb = cp.tile([P, W], dt.float32)
            nc.sync.dma_start(bslot_sb[:], bslot_d[:])
            # big resident tables
            dslot_sb = tp.tile([P, CH], dt.float32)
            nc.sync.dma_start(dslot_sb[:], dslot_d[:])

            # ---------------- phase A: h1' = dinv * (x @ W1), write ag1_in
            for b, wl in enumerate(blocks):
                nw = len(wl)
                xk = [
                    xp.tile([P, nw * P], dt.bfloat16, tag=f"x_{k}", name=f"xk_{k}")
                    for k in range(KCH)
                ]
                for k in range(KCH):
                    nc.sync.dma_start(
                        xk[k][:], x3_d[k, :, wl[0] * P : (wl[0] + nw) * P]
                    )
                for wi, w in enumerate(wl):
                    ps = psA.tile([P, HID], dt.float32)
                    for k in range(KCH):
                        nc.tensor.matmul(
                            ps[:],
                            lhsT=xk[k][:, wi * P : (wi + 1) * P],
                            rhs=w1_sb[k][:],
                            start=(k == 0),
                            stop=(k == KCH - 1),
                        )
                    h1t = ep.tile([P, HID], dt.bfloat16, tag="h1")
                    nc.vector.tensor_scalar(
                        out=h1t[:], in0=ps[:],
                        scalar1=dinvc_sb[:, w : w + 1], scalar2=None,
                        op0=mybir.AluOpType.mult,
                    )
                    nc.sync.dma_start(ag1_in[w * P : (w + 1) * P, :], h1t[:])

            # ---------------- allgather 1
            if stop != "A":
                nc.gpsimd.collective_compute(
                    "AllGather",
                    mybir.AluOpType.bypass,
                    replica_groups=rg,
                    ins=[ag1_in[:]],
                    outs=[h1_full[:]],
                )

            # ---------------- per-layer edge aggregation
            GMAX = 8
            max_nch = int(call_chunks.max())
            n_bufs_g = 2 * (-(-max_nch // GMAX))

            def agg_layer(layer, src_full):
                pool_mm = []  # (window w, out2_sb) in layer 2
                for b, wl in enumerate(blocks):
                    # gather calls for this block: per source range, split into
                    # sub-calls of <= GMAX chunks (SWDGE ring limit: 1024 idxs)
                    gt = {}
                    for r in range(NRANGE):
                        nch = int(call_chunks[b, r])
                        if nch == 0:
                            continue
                        c0 = int(call_start[b, r])
                        calls = []
                        for k in range(0, nch, GMAX):
                            n_k = min(GMAX, nch - k)
                            gix = gp.tile(
                                [P, n_k * 8], dt.int16, tag=f"gi{r}",
                                name=f"gi{r}", bufs=n_bufs_g,
                            )
                            nc.sync.dma_start(
                                gix[:],
                                gidx_d[:, (c0 + k) * 8 : (c0 + k + n_k) * 8],
                            )
                            t = gp.tile(
                                [P, n_k, HID], dt.bfloat16, tag=f"g{r}",
                                name=f"g{r}", bufs=n_bufs_g,
                            )
                            nc.gpsimd.dma_gather(
                                out_ap=t[:],
                                in_ap=src_full[
                                    r * RANGE_ROWS : (r + 1) * RANGE_ROWS, :
                                ],
                                idxs_ap=gix[:],
                                num_idxs=n_k * P,
                                num_idxs_reg=n_k * P,
                                elem_size=HID,
                            )
                            calls.append(t)
                        gt[r] = (calls, c0)
                    if part == "g":
                        continue

                    if layer == 1:
                        dvb = dp.tile([P, len(wl) * P], dt.float32, tag="dvb")
                        nc.sync.dma_start(
                            dvb[:],
                            dinvb_d[:, wl[0] * P : (wl[0] + len(wl)) * P],
                        )

                    for w in wl:
                        ncw = int(cell_chunks[w].sum())
                        ps = psAgg.tile([P, P], dt.float32)
                        i = 0
                        for r in range(NRANGE):
                            nj = int(cell_chunks[w, r])
                            if nj == 0:
                                continue
                            calls, c0 = gt[r]
                            cs = cell_start[(w, r)]
                            # one-hot S for this cell, all chunks in one DVE op
                            s_sb = sp.tile([P, nj, P], dt.bfloat16, tag="s")
                            nc.vector.tensor_tensor(
                                out=s_sb[:],
                                in0=iota_sb[:, None, :].to_broadcast([P, nj, P]),
                                in1=dslot_sb[:, cs : cs + nj, None].to_broadcast(
                                    [P, nj, P]
                                ),
                                op=mybir.AluOpType.is_equal,
                            )
                            if part == "s":
                                continue
                            for j in range(nj):
                                rel = cs - c0 + j
                                g_sl = calls[rel // GMAX][:, rel % GMAX, :]
                                if layer == 1:  # psum = G^T @ S -> [feat, slot]
                                    nc.tensor.matmul(
                                        ps[:], lhsT=g_sl, rhs=s_sb[:, j, :],
                                        start=(i == 0), stop=(i == ncw - 1),
                                    )
                                else:  # psum = S^T @ G -> [slot, feat]
                                    nc.tensor.matmul(
                                        ps[:], lhsT=s_sb[:, j, :], rhs=g_sl,
                                        start=(i == 0), stop=(i == ncw - 1),
                                    )
                                i += 1

                        if part in ("g", "s"):
                            continue
                        if part == "m":
                            h2m = ep.tile([P, HID], dt.bfloat16, tag="h2")
                            nc.vector.tensor_copy(h2m[:], ps[:])
                            nc.sync.dma_start(
                                ag2_in[w * P : (w + 1) * P, :], h2m[:]
                            )
                            continue
                        if layer == 1:
                            # out1T = relu(aggT * dinv[slot] + b1)  [feat, node]
                            dvb_w = dvb[:, (w - wl[0]) * P : (w - wl[0] + 1) * P]
                            t1 = ep.tile([P, P], dt.bfloat16, tag="t1")
                            nc.vector.tensor_tensor(
                                out=t1[:], in0=ps[:], in1=dvb_w,
                                op=mybir.AluOpType.mult,
                            )
                            o1 = ep.tile([P, P], dt.bfloat16, tag="o1")
                            nc.scalar.activation(
                                out=o1[:], in_=t1[:],
                                func=mybir.ActivationFunctionType.Relu,
                                bias=b1_sb[:],
                            )
                            # h2' = dinv[node] * (out1 @ W2)   [node, feat]
                            ps2 = psM.tile([P, HID], dt.float32)
                            nc.tensor.matmul(
                                ps2[:], lhsT=o1[:], rhs=w2_sb[:],
                                start=True, stop=True,
                            )
                            h2t = ep.tile([P, HID], dt.bfloat16, tag="h2")
                            nc.vector.tensor_scalar(
                                out=h2t[:], in0=ps2[:],
                                scalar1=dinvc_sb[:, w : w + 1], scalar2=None,
                                op0=mybir.AluOpType.mult,
                            )
                            nc.sync.dma_start(
                                ag2_in[w * P : (w + 1) * P, :], h2t[:]
                            )
                        else:
                            # out2 = relu(agg * dinv[node] + b2)  [node, feat]
                            t2 = ep.tile([P, HID], dt.bfloat16, tag="t2")
                            nc.vector.tensor_scalar(
                                out=t2[:], in0=ps[:],
                                scalar1=dinvc_sb[:, w : w + 1], scalar2=None,
                                op0=mybir.AluOpType.mult,
                            )
                            t3 = ep.tile([P, HID], dt.bfloat16, tag="t3")
                            nc.vector.tensor_tensor(
                                out=t3[:], in0=t2[:], in1=b2b_sb[:],
                                op=mybir.AluOpType.add,
                            )
                            o2 = ep.tile([P, HID], dt.bfloat16, tag="o2")
                            nc.scalar.activation(
                                out=o2[:], in_=t3[:],
                                func=mybir.ActivationFunctionType.Relu,
                            )
                            # graph one-hot
                            sg = sp.tile([P, P], dt.bfloat16, tag="sg")
                            nc.vector.tensor_scalar(
                                out=sg[:], in0=iota_sb[:],
                                scalar1=bslot_sb[:, w : w + 1], scalar2=None,
                                op0=mybir.AluOpType.is_equal,
                            )
                            pool_mm.append((w, o2, sg))
                            # pooledT += out2^T @ Sg  [feat, graph]
                            nc.tensor.matmul(
                                pool_ps[:], lhsT=o2[:], rhs=sg[:],
                                start=(w == 0), stop=(w == W - 1),
                            )
                return pool_mm

            if stop not in ("A", "AG1"):
                agg_layer(1, h1_full)

            if stop not in ("A", "AG1", "L1"):
                nc.gpsimd.collective_compute(
                    "AllGather",
                    mybir.AluOpType.bypass,
                    replica_groups=rg,
                    ins=[ag2_in[:]],
                    outs=[h2_full[:]],
                )

            pool_ps = psP.tile([P, P], dt.float32)
            if stop not in ("A", "AG1", "L1", "AG2"):
                agg_layer(2, h2_full)
            else:
                dummy = ep.tile([P, P], dt.bfloat16, tag="o2")
                nc.vector.tensor_copy(dummy[:], iota_sb[:])
                nc.tensor.matmul(pool_ps[:], lhsT=dummy[:], rhs=iota_sb[:],
                                 start=True, stop=True)

            # ---------------- final: out = (pooledT^T @ Wfc) * invcnt + bfc
            pooledT = ep.tile([P, P], dt.bfloat16, tag="pooledT")
            nc.vector.tensor_copy(pooledT[:], pool_ps[:])
            ps_fc = psM.tile([P, OUT], dt.float32, tag="ps2")
            nc.tensor.matmul(
                ps_fc[:], lhsT=pooledT[:], rhs=wfc_sb[:], start=True, stop=True
            )
            fc1 = ep.tile([P, OUT], dt.float32, tag="fc1")
            nc.vector.tensor_scalar(
                out=fc1[:], in0=ps_fc[:], scalar1=icnt_sb[:], scalar2=None,
                op0=mybir.AluOpType.mult,
            )
            fc2 = ep.tile([P, OUT], dt.float32, tag="fc2")
            nc.vector.tensor_tensor(
                out=fc2[:], in0=fc1[:], in1=bfcb_sb[:], op=mybir.AluOpType.add
            )
            nc.sync.dma_start(out_d[:], fc2[:])

    nc.compile()
    return nc


# ---------------------------------------------------------------- entry
def _run(inputs, trace=False):
    from concourse.bass_utils import run_bass_kernel_spmd

    x = np.asarray(inputs["x"], np.float32)
    meta, shared, cores = _preprocess(
        x,
        np.asarray(inputs["edge_index"]),
        np.asarray(inputs["batch"]),
        np.asarray(inputs["W1"], np.float32),
        np.asarray(inputs["b1"], np.float32),
        np.asarray(inputs["W2"], np.float32),
        np.asarray(inputs["b2"], np.float32),
        np.asarray(inputs["Wfc"], np.float32),
        np.asarray(inputs["bfc"], np.float32),
    )
    nc = _build(meta)
    in_maps = []
    for c in range(NCORES):
        cc = cores[c]
        in_maps.append(
            dict(
                x3=np.asarray(cc["x3"]),
                w1r=np.asarray(shared["w1r"]),
                w2=np.asarray(shared["w2"]),
                wfc=np.asarray(shared["wfc"]),
                b1c=shared["b1c"],
                b2b=np.asarray(shared["b2b"]),
                bfcb=shared["bfcb"],
                icnt=cc["icnt"],
                iota=np.asarray(shared["iota"]),
                dinvc=cc["dinvc"],
                dinvb=cc["dinvb"],
                bslot=cc["bslot"],
                dslot=cc["dslot"],
                gidx=cc["gidx"],
            )
        )
    res = run_bass_kernel_spmd(
        nc, in_maps, core_ids=list(range(NCORES)), trace=trace
    )
    G, OUT = meta["G"], meta["OUT"]
    gb = meta["gb"]
    out = np.zeros((G, OUT), np.float32)
    for c in range(NCORES):
        g_c = cores[c]["g_c"]
        out[gb[c] : gb[c + 1]] = res.results[c]["out"][:g_c]
    return out, res, meta


def kernel(**inputs) -> np.ndarray:
    out, _, _ = _run(inputs)
    return out


# ---------------------------------------------------------------- benchmark
def _bench(nc, in_maps, iters=12):
    """Time warm executions of the compiled NEFF with device-resident inputs.

    Returns (per-iter wall ns list, results list of per-core dicts).
    """
    import time

    import jax
    from jax.experimental.shard_map import shard_map
    from jax.sharding import Mesh, NamedSharding, PartitionSpec

    import concourse.mybir as mybir
    from concourse import bass2jax

    bass2jax.install_neuronx_cc_hook()
    n_cores = len(in_maps)

    partition_name = (
        nc.partition_id_tensor.name if nc.partition_id_tensor else None
    )
    in_names, out_names, out_avals, zero_outs = [], [], [], []
    for alloc in nc.m.functions[0].allocations:
        if not isinstance(alloc, mybir.MemoryLocationSet):
            continue
        name = alloc.memorylocations[0].name
        if alloc.kind == "ExternalInput":
            if name != partition_name:
                in_names.append(name)
        elif alloc.kind == "ExternalOutput":
            out_names.append(name)
            shape = tuple(alloc.tensor_shape)
            dtype = mybir.dt.np(alloc.dtype)
            out_avals.append(jax.core.ShapedArray(shape, dtype))
            zero_outs.append(np.zeros(shape, dtype))
    n_params = len(in_names)
    all_names = in_names + out_names
    if partition_name is not None:
        all_names = all_names + [partition_name]

    def _body(*args):
        operands = list(args)
        if partition_name is not None:
            operands.append(bass2jax.partition_id_tensor())
        outs = bass2jax._bass_exec_p.bind(
            *operands,
            out_avals=tuple(out_avals),
            in_names=tuple(all_names),
            out_names=tuple(out_names),
            lowering_input_output_aliases=(),
            sim_require_finite=True,
            sim_require_nnan=True,
            nc=nc,
        )
        return tuple(outs)

    devices = jax.devices()[:n_cores]
    mesh = Mesh(np.asarray(devices), ("core",))
    spec = PartitionSpec("core")
    n_outs = len(out_names)
    donate = tuple(range(n_params, n_params + n_outs))
    sharded = jax.jit(
        shard_map(
            _body,
            mesh=mesh,
            in_specs=(spec,) * (n_params + n_outs),
            out_specs=(spec,) * n_outs,
            check_rep=False,
        ),
        donate_argnums=donate,
        keep_unused=True,
    )
    sh = NamedSharding(mesh, spec)
    dev_in = [
        jax.device_put(
            np.concatenate([np.asarray(m[name]) for m in in_maps], axis=0), sh
        )
        for name in in_names
    ]
    concat_zeros = [
        np.zeros((n_cores * z.shape[0], *z.shape[1:]), z.dtype) for z in zero_outs
    ]

    out_arrs = None
    times = []
    for it in range(iters):
        zs = [jax.device_put(z, sh) for z in concat_zeros]
        jax.block_until_ready(zs)
        t0 = time.perf_counter()
        res = sharded(*dev_in, *zs)
        jax.block_until_ready(res)
        t1 = time.perf_counter()
        times.append((t1 - t0) * 1e9)
        if out_arrs is None:
            out_arrs = [np.asarray(a) for a in res]
        else:
            for a in res:
                a.delete()
    results = [
        {
            name: out_arrs[i].reshape(n_cores, *out_avals[i].shape)[c]
            for i, name in enumerate(out_names)
        }
        for c in range(n_cores)
    ]
    return times, results


def _run_bench(inputs, iters=12):
    x = np.asarray(inputs["x"], np.float32)
    meta, shared, cores = _preprocess(
        x,
        np.asarray(inputs["edge_index"]),
        np.asarray(inputs["batch"]),
        np.asarray(inputs["W1"], np.float32),
        np.asarray(inputs["b1"], np.float32),
        np.asarray(inputs["W2"], np.float32),
        np.asarray(inputs["b2"], np.float32),
        np.asarray(inputs["Wfc"], np.float32),
        np.asarray(inputs["bfc"], np.float32),
    )
    nc = _build(meta)
    in_maps = []
    for c in range(NCORES):
        cc = cores[c]
        in_maps.append(
            dict(
                x3=np.asarray(cc["x3"]),
                w1r=np.asarray(shared["w1r"]),
                w2=np.asarray(shared["w2"]),
                wfc=np.asarray(shared["wfc"]),
                b1c=shared["b1c"],
                b2b=np.asarray(shared["b2b"]),
                bfcb=shared["bfcb"],
                icnt=cc["icnt"],
                iota=np.asarray(shared["iota"]),
                dinvc=cc["dinvc"],
                dinvb=cc["dinvb"],
                bslot=cc["bslot"],
                dslot=cc["dslot"],
                gidx=cc["gidx"],
            )
        )
    times, results = _bench(nc, in_maps, iters=iters)
    G, OUT = meta["G"], meta["OUT"]
    gb = meta["gb"]
    out = np.zeros((G, OUT), np.float32)
    for c in range(NCORES):
        g_c = cores[c]["g_c"]
        out[gb[c] : gb[c + 1]] = results[c]["out"][:g_c]
    return out, times, meta
